# revision 1
# baseline (speedup 1.0000x reference)
"""Trainium2 Bass kernel for nn_PolicyNet_78365973283198 (GNN message passing).

Computation (reference):
    tempHS = tanh(state_HS @ W_fs + b_fs)          # [N, 128]
    u0     = tempHS @ W_fp + b_fp                  # [N]
    uk[e]  = <tempHS[seg[e]], hats[e]>             # [E]  (seg sorted)
    out    = sigmoid(concat([u0, uk]))             # [N + E]

Design: data-parallel over nodes on 8 cores. Each node's edge list is split
on host into fixed-size chunks (classes D in {8, 16, 32}); chunk rows (with
state_HS rows duplicated per chunk) are packed into 128-row blocks, so the
ragged gather tempHS[seg] becomes matmuls against CONSTANT selection
matrices (no per-edge one-hot is ever built on device -- that was the old
bottleneck: GpSimd is_equal at 97% occupancy).

Per block: phase A computes tempHS^T with W-stationary matmuls ([d, n],
b_fs folded into the per-partition ACT tanh bias); u0 is one N=1 matmul
(lhsT=thT, rhs=W_fp col) into a PSUM bank column; a PE transpose gives
th [n, d]. Per 4-tile edge group: ONE wide matmul ep4 = th.T @ S' (S' a
constant 0/1 selection [n, 512]) expands chunk rows to edge columns in
[d, e] orientation; a DVE tensor_tensor multiplies by the hats tile
(packed [d, e] on host, fp16); and a PE matmul with a zeros|ones column
slice (f32r moving operand: full speed, near-f32 precision) reduces over
d, accumulating group g's 512 edge dots into row g of a PSUM uk bank.
uk is DMA'd out pre-sigmoid (host applies sigmoid during unpack); u0 is
sigmoided on device. Bulk state/hats loads are batched 8 blocks/groups
per DMA to amortize the ~565ns/DMA sequencer cost.
"""

import numpy as np

N_NODES = 50000
N_EDGES = 600000
IN_DIM = 512
E_DIM = 128
NC = 8
NPER = N_NODES // NC
CLASSES = (8, 16, 32)
HBATCH = 8          # hats groups per DMA
SBATCH = 8          # state blocks per DMA


def _f16(x):
    return np.ascontiguousarray(x, dtype=np.float16)


def _decompose(deg):
    d = deg
    n8 = ((d <= 8) | ((d > 16) & (d <= 24))).astype(np.int64)
    n16 = (((d > 8) & (d <= 24))).astype(np.int64)
    n32 = (d > 24).astype(np.int64)
    assert (d <= 32).all(), "degree > 32 unsupported by class decomposition"
    return n8, n16, n32


def _geometry(nblocks):
    B = sum(nblocks[D] for D in CLASSES)
    G = sum(nblocks[D] * (D // 4) for D in CLASSES)
    GP = -(-G // HBATCH) * HBATCH                  # padded group count
    SLOTS = sum(nblocks[D] * 128 * D for D in CLASSES)
    return B, G, GP, SLOTS


def emit(nc, t, nblocks, copy_cycle=None):
    """Emit the program body. t = dict name -> DRAM AP/handle."""
    import concourse.bass as bass
    import concourse.tile as tile
    from concourse import mybir

    fp16 = mybir.dt.float16
    f32 = mybir.dt.float32
    f32r = mybir.dt.float32r
    Act = mybir.ActivationFunctionType

    B, G, GP, SLOTS = _geometry(nblocks)
    NBANK = -(-G // 128)
    assert NBANK <= 2
    wcat_d, bfs_d, wfp_d, bfp_d = t["wcat"], t["bfs"], t["wfp"], t["bfp"]
    state_d, hats_d = t["state_p"], t["hats_p"]
    sgrp_d, idn_d, zo_d = t["sgrp"], t["idn"], t["zo"]
    uk_d, u0_d = t["uk_o"], t["u0_o"]
    NSG = sum(D // 4 for D in CLASSES)          # S' variants (14)

    with tile.TileContext(nc) as tc:
        with (
            tc.tile_pool(name="const", bufs=1) as cpool,
            tc.tile_pool(name="perst", bufs=1) as ppool,
            tc.tile_pool(name="st", bufs=3) as stpool,
            tc.tile_pool(name="th", bufs=4) as thpool,
            tc.tile_pool(name="hat", bufs=4) as hpool,
            tc.tile_pool(name="pp", bufs=6) as ppool2,
            tc.tile_pool(name="psA", bufs=1, space="PSUM") as psA,
            tc.tile_pool(name="psT", bufs=1, space="PSUM") as psT,
            tc.tile_pool(name="psC", bufs=3, space="PSUM") as psC,
            tc.tile_pool(name="psK", bufs=1, space="PSUM") as psK,
            tc.tile_pool(name="psU", bufs=1, space="PSUM") as psU,
        ):
            wcat = cpool.tile([128, 8 * 128], fp16, tag="wcat")
            nc.sync.dma_start(wcat[:], wcat_d[:])
            bfs = cpool.tile([128, 1], fp16, tag="bfs")
            nc.sync.dma_start(bfs[:], bfs_d[:])
            wfp = cpool.tile([128, 1], fp16, tag="wfp")
            nc.sync.dma_start(wfp[:], wfp_d[:])
            bfp = cpool.tile([128, 1], f32, tag="bfp")
            nc.sync.dma_start(bfp[:], bfp_d[:])
            idn = cpool.tile([128, 128], fp16, tag="idn")
            nc.sync.dma_start(idn[:], idn_d[:])
            zo = cpool.tile([128, 512], f32r, tag="zo")
            nc.sync.dma_start(zo[:], zo_d.bitcast(f32r) if zo_d.dtype != f32r else zo_d)
            sgrp = cpool.tile([128, NSG * 512], fp16, tag="sgrp")
            nc.sync.dma_start(sgrp[:], sgrp_d[:])

            u0acc = psU.tile([128, B], f32, tag="u0acc")
            ukb = [psK.tile([128, 512], f32, tag=f"ukb{b}", name=f"ukb{b}")
                   for b in range(NBANK)]
            # init uk banks to zero via all-zero stationary matmuls
            for b in range(NBANK):
                nc.tensor.matmul(ukb[b][:], lhsT=zo[:, 0:128], rhs=zo[:],
                                 start=True, stop=False,
                                 skip_group_check=True)

            st_bufs, hat_bufs = {}, {}
            g = 0
            kk = 0
            sg0 = 0                               # S' variant base per class
            for D in CLASSES:
                for _k in range(nblocks[D]):
                    sb, sj = divmod(kk, SBATCH)
                    if sj == 0:
                        stb = stpool.tile([128, SBATCH * 512], fp16, tag="st")
                        nc.sync.dma_start(stb[:], state_d[sb])
                        st_bufs[sb] = stb
                    stb = st_bufs[sb]
                    tpT = psA.tile([128, 128], f32, tag="tpT")
                    for i in range(8):          # (hl, c4) combos
                        c4 = i % 4
                        nc.tensor.matmul(
                            tpT[:],
                            lhsT=wcat[:, i * 128:(i + 1) * 128],
                            rhs=stb[:, sj * 512 + c4 * 128:
                                    sj * 512 + (c4 + 1) * 128],
                            start=(i == 0), stop=(i == 7),
                        )
                    thT = thpool.tile([128, 128], fp16, tag="thT")
                    nc.scalar.activation(thT[:], tpT[:], Act.Tanh,
                                         bias=bfs[:, 0:1])
                    nc.tensor.matmul(u0acc[:, kk:kk + 1], lhsT=thT[:],
                                     rhs=wfp[:], start=True, stop=True)
                    # transpose thT -> th [n, d]
                    thp = psT.tile([128, 128], f32, tag="thp")
                    nc.tensor.matmul(thp[:], lhsT=thT[:], rhs=idn[:],
                                     start=True, stop=True)
                    th = thpool.tile([128, 128], fp16, tag="th")
                    nc.scalar.activation(th[:], thp[:], Act.Copy)
                    for gb in range(D // 4):
                        hb, hj = divmod(g, HBATCH)
                        if hj == 0:
                            hatb = hpool.tile([128, HBATCH * 512], fp16,
                                              tag="hat")
                            nc.scalar.dma_start(hatb[:], hats_d[hb])
                            hat_bufs[hb] = hatb
                        hatb = hat_bufs[hb]
                        ep4 = psC.tile([128, 512], f32, tag="ep4")
                        sg = sg0 + gb
                        nc.tensor.matmul(ep4[:], lhsT=th[:],
                                         rhs=sgrp[:, sg * 512:(sg + 1) * 512],
                                         start=True, stop=True)
                        P4 = ppool2.tile([128, 512], f32r, tag="P4")
                        nc.vector.tensor_tensor(
                            out=P4[:], in0=ep4[:],
                            in1=hatb[:, hj * 512:(hj + 1) * 512],
                            op=mybir.AluOpType.mult)
                        bank, row = divmod(g, 128)
                        last = (g == G - 1) or (g == 127)
                        nc.tensor.matmul(
                            ukb[bank][0:row + 1, :],
                            lhsT=zo[:, 128 - row:129], rhs=P4[:],
                            start=False, stop=last, skip_group_check=True)
                        g += 1
                    kk += 1
                sg0 += D // 4

            for b in range(NBANK):
                uks = ppool.tile([128, 512], f32, tag=f"uks{b}")
                nc.scalar.activation(uks[:], ukb[b][:], Act.Copy)
                nc.sync.dma_start(uk_d[b], uks[:])
            u0sb = ppool.tile([128, B], f32, tag="u0sb")
            nc.scalar.activation(u0sb[:], u0acc[:], Act.Sigmoid,
                                 bias=bfp[:, 0:1])
            nc.sync.dma_start(u0_d[:], u0sb[:])
    return []


def build_nc(nblocks, copy_cycle=None):
    import concourse.bass as bass
    from concourse import mybir

    fp16 = mybir.dt.float16
    f32 = mybir.dt.float32
    B, G, GP, SLOTS = _geometry(nblocks)
    NSB = -(-B // SBATCH)
    NBANK = -(-G // 128)
    NSG = sum(D // 4 for D in CLASSES)

    nc = bass.Bass("TRN2", target_bir_lowering=False, debug=False)
    t = {
        "wcat": nc.dram_tensor("wcat", [128, 8 * 128], fp16,
                               kind="ExternalInput")[:],
        "bfs": nc.dram_tensor("bfs", [128, 1], fp16, kind="ExternalInput")[:],
        "wfp": nc.dram_tensor("wfp", [128, 1], fp16, kind="ExternalInput")[:],
        "bfp": nc.dram_tensor("bfp", [128, 1], f32, kind="ExternalInput")[:],
        "idn": nc.dram_tensor("idn", [128, 128], fp16,
                              kind="ExternalInput")[:],
        "zo": nc.dram_tensor("zo", [128, 512], mybir.dt.float32r, kind="ExternalInput")[:],
        "sgrp": nc.dram_tensor("sgrp", [128, NSG * 512], fp16,
                               kind="ExternalInput")[:],
        "state_p": nc.dram_tensor("state_p", [NSB, 128, SBATCH * 512], fp16,
                                  kind="ExternalInput"),
        "hats_p": nc.dram_tensor("hats_p", [GP // HBATCH, 128, HBATCH * 512],
                                 fp16, kind="ExternalInput"),
        "uk_o": nc.dram_tensor("uk_o", [NBANK, 128, 512], f32,
                               kind="ExternalOutput"),
        "u0_o": nc.dram_tensor("u0_o", [128, B], f32,
                               kind="ExternalOutput")[:],
    }
    emit(nc, t, nblocks, copy_cycle)
    split_multi_waits(nc)
    return nc


def split_multi_waits(nc):
    """This env's walrus encodes at most one sem wait per instruction; hoist
    extras onto standalone EventSemaphore insts immediately before."""
    import concourse.mybir as mybir
    n = 0
    for fn in nc.m.functions:
        for bb in fn.blocks:
            insts = list(bb.instructions)
            if not any(i.sync_info and len(i.sync_info.on_wait) > 1 for i in insts):
                continue
            out = []
            for inst in insts:
                si = inst.sync_info
                if si is not None and len(si.on_wait) > 1:
                    waits = list(si.on_wait)
                    for w in waits[:-1]:
                        n += 1
                        out.append(mybir.InstEventSemaphore(
                            name=f"splitw_{n}_{inst.name}",
                            engine=inst.engine, ins=[], outs=[],
                            sync_info=mybir.SyncInfo(on_wait=[w], on_update=[]),
                        ))
                    inst.sync_info = mybir.SyncInfo(
                        on_wait=[waits[-1]], on_update=list(si.on_update))
                out.append(inst)
            bb.instructions = out
    return n


def prep_inputs(state_HS, hats, seg, W_fs, b_fs, W_fp, b_fp):
    """Shard + pack. Returns (in_maps, nblocks, maps)."""
    state_HS = np.asarray(state_HS, dtype=np.float32)
    hats = np.asarray(hats, dtype=np.float32)
    seg = np.asarray(seg, dtype=np.int32)
    W_fs = np.asarray(W_fs, dtype=np.float32)
    b_fs = np.asarray(b_fs, dtype=np.float32)
    W_fp = np.asarray(W_fp, dtype=np.float32)
    b_fp = np.asarray(b_fp, dtype=np.float32)

    deg = np.bincount(seg, minlength=N_NODES).astype(np.int64)
    estart = np.concatenate([[0], np.cumsum(deg)[:-1]])
    n8, n16, n32 = _decompose(deg)
    cnt = {8: n8, 16: n16, 32: n32}

    nblocks = {}
    for D in CLASSES:
        percore = cnt[D].reshape(NC, NPER).sum(1)
        nblocks[D] = int(-(-percore.max() // 128))
    B, G, GP, SLOTS = _geometry(nblocks)
    NSB = -(-B // SBATCH)

    w_hi = W_fs.astype(np.float16)
    w_lo = (W_fs.astype(np.float64) - w_hi.astype(np.float64)).astype(np.float16)
    wcat = np.empty((128, 8, 128), dtype=np.float16)
    for hl, w in enumerate((w_hi, w_lo)):
        for c4 in range(4):
            wcat[:, hl * 4 + c4, :] = w[c4 * 128:(c4 + 1) * 128, :]
    wcat = wcat.reshape(128, 8 * 128)
    bfs_c = _f16(b_fs.reshape(128, 1))
    wfp_c = _f16(W_fp.reshape(128, 1))
    bfp_c = np.full((128, 1), float(b_fp[0]), dtype=np.float32)
    idn_c = _f16(np.eye(128, dtype=np.float32))
    zo_c = np.zeros((128, 512), np.float32)
    zo_c[:, 128] = 1.0
    # S' selection constants: per class D, per group-in-block gb:
    # sgrp[n, (sg)*512 + tl*128 + q*D + j] = (n == (4*gb+tl)*(128//D) + q)
    sgs = []
    for D in CLASSES:
        r = 128 // D
        for gb in range(D // 4):
            s = np.zeros((128, 4, r, D), np.float16)
            for tl in range(4):
                t_abs = 4 * gb + tl
                for q in range(r):
                    s[t_abs * r + q, tl, q, :] = 1.0
            sgs.append(s.reshape(128, 512))
    sgrp_c = np.ascontiguousarray(np.concatenate(sgs, axis=1))

    state16 = state_HS.astype(np.float16)
    hats16 = hats.astype(np.float16)

    in_maps, maps = [], []
    for c in range(NC):
        nlo = c * NPER
        nodes = np.arange(nlo, nlo + NPER)
        st_blocks = []
        hat_groups = np.zeros((GP, 128, 512), np.float16)
        cls_info = []
        u0_pos = np.full(NPER, -1, np.int64)
        kk0 = 0
        g0 = 0
        for D in CLASSES:
            r = 128 // D
            nb = nblocks[D]
            nch = nb * 128
            sel = nodes[cnt[D][nodes] > 0]
            chunk_node = np.full(nch, nlo, np.int64)
            chunk_node[:len(sel)] = sel
            if D == 8:
                off_in_node = np.where(deg[chunk_node] > 16, 16, 0)
            else:
                off_in_node = np.zeros(nch, np.int64)
            ce_start = estart[chunk_node] + off_in_node
            ce_len = np.minimum(deg[chunk_node] - off_in_node, D)
            ce_len[len(sel):] = 0
            ce_len = np.maximum(ce_len, 0)
            arr = state16[chunk_node].reshape(nb, 128, 4, 128)
            st_blocks.append(
                np.ascontiguousarray(arr.transpose(0, 3, 2, 1))
                .reshape(nb, 128, 512))
            sl = ce_start[:, None] + np.arange(D)[None, :]
            valid = np.arange(D)[None, :] < ce_len[:, None]
            slot_edge = np.where(valid, sl, -1)
            hv = np.zeros((nch, D, E_DIM), np.float16)
            hv[valid] = hats16[slot_edge[valid]]
            # rows -> (t, q); groups gb of 4 tiles; layout [g, d, tl, q, j]
            hv = hv.reshape(nb, D // 4, 4, r, D, E_DIM)
            hv = hv.transpose(0, 1, 5, 2, 3, 4).reshape(nb * (D // 4), 128, 512)
            hat_groups[g0:g0 + nb * (D // 4)] = hv
            cls_info.append((D, slot_edge, g0))
            rows = kk0 * 128 + np.arange(len(sel))
            mask = u0_pos[sel - nlo] < 0
            u0_pos[(sel - nlo)[mask]] = rows[mask]
            kk0 += nb
            g0 += nb * (D // 4)
        st_p = np.concatenate(st_blocks)                  # [B, 128, 512]
        # re-batch: [NSB, 128, SBATCH*512]
        st_b = np.zeros((NSB * SBATCH, 128, 512), np.float16)
        st_b[:B] = st_p
        st_b = (st_b.reshape(NSB, SBATCH, 128, 512)
                .transpose(0, 2, 1, 3).reshape(NSB, 128, SBATCH * 512))
        hp_b = (hat_groups.reshape(GP // HBATCH, HBATCH, 128, 512)
                .transpose(0, 2, 1, 3).reshape(GP // HBATCH, 128, HBATCH * 512))
        in_maps.append({
            "wcat": wcat, "bfs": bfs_c, "wfp": wfp_c, "bfp": bfp_c,
            "idn": idn_c, "zo": zo_c, "sgrp": sgrp_c,
            "state_p": np.ascontiguousarray(st_b),
            "hats_p": np.ascontiguousarray(hp_b),
        })
        maps.append((cls_info, u0_pos))
    return in_maps, nblocks, maps


def assemble(results, nblocks, maps):
    out = np.empty(N_NODES + N_EDGES, dtype=np.float32)
    for c in range(NC):
        cls_info, u0_pos = maps[c]
        uk = np.asarray(results[c]["uk_o"]).reshape(-1, 512)  # [NBANK*128,512]
        u0 = np.asarray(results[c]["u0_o"])                   # [128, B]
        p = u0_pos
        out[c * NPER:(c + 1) * NPER] = u0[p % 128, p // 128]
        for (D, slot_edge, g0) in cls_info:
            nch = slot_edge.shape[0]
            r = 128 // D
            ch = np.arange(nch)[:, None]
            j = np.arange(D)[None, :]
            k = ch // 128
            s = ch % 128
            t = s // r
            q = s % r
            gb = t // 4
            tl = t % 4
            gg = g0 + k * (D // 4) + gb
            e512 = tl * 128 + q * D + j
            valid = slot_edge >= 0
            gg_b = np.broadcast_to(gg, valid.shape)
            vals = uk[gg_b[valid], e512[valid]]
            out[N_NODES + slot_edge[valid]] = 1.0 / (1.0 + np.exp(-vals))
    return out


def kernel(state_HS, hats, seg, W_fs, b_fs, W_fp, b_fp):
    from concourse.bass_utils import run_bass_kernel_spmd
    in_maps, nblocks, maps = prep_inputs(
        state_HS, hats, seg, W_fs, b_fs, W_fp, b_fp)
    nc = build_nc(nblocks)
    res = run_bass_kernel_spmd(nc, in_maps, core_ids=list(range(NC)))
    return assemble(res.results, nblocks, maps)



# revision 4
# speedup vs baseline: 1.0518x; 1.0518x over previous
"""Trainium2 Bass kernel for nn_PolicyNet_78365973283198 (GNN message passing).

Computation (reference):
    tempHS = tanh(state_HS @ W_fs + b_fs)          # [N, 128]
    u0     = tempHS @ W_fp + b_fp                  # [N]
    uk[e]  = <tempHS[seg[e]], hats[e]>             # [E]  (seg sorted)
    out    = sigmoid(concat([u0, uk]))             # [N + E]

Design: data-parallel over nodes on 8 cores (6250 nodes each). Nodes are
sorted by degree (descending); the degree sequence is canonicalized to the
per-rank max across cores so ONE program serves all 8 cores (~4% pad).
Blocks of 128 chunks share a uniform degree d_b (block max).

uk via ALL-PAIRS matmuls: per block, phase A produces thT [d=128, n=128]
(fp16, W hi/lo split for precision). For each group of C chunks
(C = 32/16/8 so W = C*d <= 512), ONE matmul out[c, s] = <th[g*C+c], hats[s]>
against the RAW hats tile (packed [E_DIM, slots] fp16 on host) computes every
needed dot product at 1 PE cycle/slot -- no expansion matmul, no DVE multiply,
no reduce matmul (the old pipeline cost ~2x PE + a DVE pass). The needed
values form a per-group diagonal band; Act/DVE copy the PSUM group tiles into
an SBUF stage and a single 3-dim "diagonal" DMA per block
([(L+d, C), (W, ng), (1, d)] -- partition-crossing stride on dim 0 only,
offset < row length, C <= 42: all hardware-validated) extracts the band
straight to DRAM. u0 rides on the same loaded weights as an N=1 matmul.
Host applies sigmoid to uk during unpack; u0 is sigmoided on device.
"""

import numpy as np

N_NODES = 50000
N_EDGES = 600000
IN_DIM = 512
E_DIM = 128
NC = 8
NPER = N_NODES // NC
SEGCOLS = 8192          # hats segment width (cols) = 2MB fp16 per DMA
HB_BUFS = 4             # hats segment ring depth
STGW = 4608             # uk stage width (f32 cols); caps block degree at 36


def _f16(x):
    return np.ascontiguousarray(x, dtype=np.float16)


def _group_c(d):
    if d <= 16:
        return 32
    if d <= 32:
        return 16
    if d <= 64:
        return 8
    raise AssertionError(f"degree {d} > 64 unsupported")


class Geom:
    """Canonical (core-independent) program geometry."""

    def __init__(self, D):
        # D: canonical per-chunk degrees, len NPER, sorted descending
        NBLK = -(-NPER // 128)
        self.NR = -(-NBLK // 4)
        self.NBLK4 = 4 * self.NR
        self.NCHUNK = 128 * self.NBLK4
        Dp = np.zeros(self.NCHUNK, np.int64)
        Dp[:NPER] = D
        self.d_b = [int(Dp[128 * b]) for b in range(self.NBLK4)]
        assert 128 * max(self.d_b) <= STGW, f"max block degree {max(self.d_b)}"

        self.blocks = []        # per block: None or (d, C, ng, W, [(seg, off)]*ng, ukoff)
        seg_i, cur, ukoff = 0, 0, 0
        for b in range(self.NBLK4):
            d = self.d_b[b]
            if d == 0:
                self.blocks.append(None)
                continue
            C = _group_c(d)
            ng = 128 // C
            W = C * d
            gplace = []
            for g in range(ng):
                if cur + W > SEGCOLS:
                    seg_i += 1
                    cur = 0
                gplace.append((seg_i, cur))
                cur += W
            self.blocks.append((d, C, ng, W, gplace, ukoff))
            ukoff += 128 * d
        self.NSEG = seg_i + 1
        self.SUK = max(ukoff, 1)


def emit(nc, t, geom):
    import concourse.tile as tile
    from concourse import mybir
    from concourse.ap import AP

    fp16 = mybir.dt.float16
    f32 = mybir.dt.float32
    Act = mybir.ActivationFunctionType

    wcat_d, bfs_d, wfp_d, bfp_d = t["wcat"], t["bfs"], t["wfp"], t["bfp"]
    state_d, hats_d = t["state_p"], t["hats_p"]
    uk_d, u0_d = t["uk_o"], t["u0_o"]
    NR, NSEG, NBLK4 = geom.NR, geom.NSEG, geom.NBLK4

    with tile.TileContext(nc) as tc:
        with (
            tc.tile_pool(name="const", bufs=1) as cpool,
            tc.tile_pool(name="perst", bufs=1) as ppool,
            tc.tile_pool(name="st", bufs=3) as stpool,
            tc.tile_pool(name="th", bufs=3) as thpool,
            tc.tile_pool(name="hat", bufs=HB_BUFS) as hpool,
            tc.tile_pool(name="stg", bufs=2) as sgpool,
            tc.tile_pool(name="psA", bufs=2, space="PSUM") as psA,
            tc.tile_pool(name="psK", bufs=5, space="PSUM") as psK,
            tc.tile_pool(name="psU", bufs=1, space="PSUM") as psU,
        ):
            wcat = cpool.tile([128, 8 * 128], fp16, tag="wcat")
            nc.sync.dma_start(wcat[:], wcat_d[:])
            bfs = cpool.tile([128, 1], fp16, tag="bfs")
            nc.sync.dma_start(bfs[:], bfs_d[:])
            wfp = cpool.tile([128, 1], fp16, tag="wfp")
            nc.sync.dma_start(wfp[:], wfp_d[:])
            bfp = cpool.tile([128, 1], f32, tag="bfp")
            nc.sync.dma_start(bfp[:], bfp_d[:])

            u0acc = psU.tile([128, NBLK4], f32, tag="u0acc")

            # hats segment ring with explicit prefetch emission
            hseg = {}
            next_seg = [0]

            def fetch_seg():
                s = next_seg[0]
                if s >= NSEG:
                    return
                tile_ = hpool.tile([128, SEGCOLS], fp16, tag="hseg")
                eng = nc.sync if s % 2 == 0 else nc.scalar
                eng.dma_start(tile_[:], hats_d[s])
                hseg[s] = tile_
                next_seg[0] += 1

            for _ in range(min(HB_BUFS - 1, NSEG)):
                fetch_seg()

            st_tiles = {}
            next_st = [0]

            def fetch_st():
                r = next_st[0]
                if r >= NR:
                    return
                tile_ = stpool.tile([128, 2048], fp16, tag="st")
                eng = nc.scalar if r % 2 == 0 else nc.sync
                eng.dma_start(tile_[:], state_d[r])
                st_tiles[r] = tile_
                next_st[0] += 1

            fetch_st()
            fetch_st()

            cp_i = [0]          # copy engine round robin

            def copy(out_ap, in_ap):
                # weight DVE slightly more than Act (Act also does tanh+DMA)
                k = cp_i[0] % 5
                cp_i[0] += 1
                if k in (0, 1, 3):
                    nc.vector.tensor_scalar_add(out=out_ap, in0=in_ap, scalar1=0.0)
                else:
                    nc.scalar.activation(out_ap, in_ap, Act.Copy)

            for r in range(NR):
                stb = st_tiles.pop(r)
                fetch_st()
                tp = psA.tile([128, 512], f32, tag="tp")
                for i in range(8):          # (hi/lo, c4) combos
                    c4 = i % 4
                    nc.tensor.matmul(
                        tp[:],
                        lhsT=wcat[:, i * 128:(i + 1) * 128],
                        rhs=stb[:, c4 * 512:(c4 + 1) * 512],
                        start=(i == 0), stop=(i == 7),
                    )
                thT4 = thpool.tile([128, 512], fp16, tag="thT4")
                nc.scalar.activation(thT4[:], tp[:], Act.Tanh, bias=bfs[:, 0:1])

                for b4 in range(4):
                    b = 4 * r + b4
                    thT = thT4[:, b4 * 128:(b4 + 1) * 128]
                    nc.tensor.matmul(u0acc[:, b:b + 1], lhsT=thT, rhs=wfp[:],
                                     start=True, stop=True)
                    blk = geom.blocks[b]
                    if blk is None:
                        continue
                    d, C, ng, W, gplace, ukoff = blk
                    LB = ng * W     # = 128*d
                    stage = sgpool.tile([128, STGW], f32, tag="stage")
                    for g in range(ng):
                        s, off = gplace[g]
                        while next_seg[0] <= s + (HB_BUFS - 2):
                            if next_seg[0] >= NSEG:
                                break
                            fetch_seg()
                        hs = hseg[s]
                        pk = psK.tile([128, 512], f32, tag="pk")
                        nc.tensor.matmul(
                            pk[0:C, 0:W],
                            lhsT=thT[:, g * C:(g + 1) * C],
                            rhs=hs[:, off:off + W],
                            start=True, stop=True)
                        copy(stage[0:C, g * W:(g + 1) * W], pk[0:C, 0:W])
                    sap = stage[:]
                    diag = AP(sap.tensor, sap.offset,
                              [(STGW + d, C), (W, ng), (1, d)])
                    nc.sync.dma_start(uk_d[0:1, ukoff:ukoff + 128 * d], diag)

            u0sb = ppool.tile([128, NBLK4], f32, tag="u0sb")
            nc.scalar.activation(u0sb[:], u0acc[:], Act.Sigmoid, bias=bfp[:, 0:1])
            nc.sync.dma_start(u0_d[:], u0sb[:])
    return []


def build_nc(geom):
    import concourse.bass as bass
    from concourse import mybir

    fp16 = mybir.dt.float16
    f32 = mybir.dt.float32

    nc = bass.Bass("TRN2", target_bir_lowering=False, debug=False)
    t = {
        "wcat": nc.dram_tensor("wcat", [128, 8 * 128], fp16,
                               kind="ExternalInput")[:],
        "bfs": nc.dram_tensor("bfs", [128, 1], fp16, kind="ExternalInput")[:],
        "wfp": nc.dram_tensor("wfp", [128, 1], fp16, kind="ExternalInput")[:],
        "bfp": nc.dram_tensor("bfp", [128, 1], f32, kind="ExternalInput")[:],
        "state_p": nc.dram_tensor("state_p", [geom.NR, 128, 2048], fp16,
                                  kind="ExternalInput"),
        "hats_p": nc.dram_tensor("hats_p", [geom.NSEG, 128, SEGCOLS], fp16,
                                 kind="ExternalInput"),
        "uk_o": nc.dram_tensor("uk_o", [1, geom.SUK], f32,
                               kind="ExternalOutput")[:],
        "u0_o": nc.dram_tensor("u0_o", [128, geom.NBLK4], f32,
                               kind="ExternalOutput")[:],
    }
    emit(nc, t, geom)
    split_multi_waits(nc)
    return nc


def split_multi_waits(nc):
    """This env's walrus encodes at most one sem wait per instruction; hoist
    extras onto standalone EventSemaphore insts immediately before."""
    import concourse.mybir as mybir
    n = 0
    for fn in nc.m.functions:
        for bb in fn.blocks:
            insts = list(bb.instructions)
            if not any(i.sync_info and len(i.sync_info.on_wait) > 1 for i in insts):
                continue
            out = []
            for inst in insts:
                si = inst.sync_info
                if si is not None and len(si.on_wait) > 1:
                    waits = list(si.on_wait)
                    for w in waits[:-1]:
                        n += 1
                        out.append(mybir.InstEventSemaphore(
                            name=f"splitw_{n}_{inst.name}",
                            engine=inst.engine, ins=[], outs=[],
                            sync_info=mybir.SyncInfo(on_wait=[w], on_update=[]),
                        ))
                    inst.sync_info = mybir.SyncInfo(
                        on_wait=[waits[-1]], on_update=list(si.on_update))
                out.append(inst)
            bb.instructions = out
    return n


def prep_inputs(state_HS, hats, seg, W_fs, b_fs, W_fp, b_fp):
    """Shard + pack. Returns (in_maps, geom, maps)."""
    state_HS = np.asarray(state_HS, dtype=np.float32)
    hats = np.asarray(hats, dtype=np.float32)
    seg = np.asarray(seg, dtype=np.int32)
    W_fs = np.asarray(W_fs, dtype=np.float32)
    b_fs = np.asarray(b_fs, dtype=np.float32)
    W_fp = np.asarray(W_fp, dtype=np.float32)
    b_fp = np.asarray(b_fp, dtype=np.float32)

    deg = np.bincount(seg, minlength=N_NODES).astype(np.int64)
    estart = np.concatenate([[0], np.cumsum(deg)[:-1]])

    orders = []
    deg_sorted = np.empty((NC, NPER), np.int64)
    for c in range(NC):
        nodes = np.arange(c * NPER, (c + 1) * NPER)
        o = np.lexsort((nodes, -deg[nodes]))
        orders.append(nodes[o])
        deg_sorted[c] = deg[nodes[o]]
    D = deg_sorted.max(axis=0)
    geom = Geom(D)

    # constants
    w_hi = W_fs.astype(np.float16)
    w_lo = (W_fs.astype(np.float64) - w_hi.astype(np.float64)).astype(np.float16)
    wcat = np.empty((128, 8, 128), dtype=np.float16)
    for hl, w in enumerate((w_hi, w_lo)):
        for c4 in range(4):
            wcat[:, hl * 4 + c4, :] = w[c4 * 128:(c4 + 1) * 128, :]
    wcat = wcat.reshape(128, 8 * 128)
    bfs_c = _f16(b_fs.reshape(128, 1))
    wfp_c = _f16(W_fp.reshape(128, 1))
    bfp_c = np.full((128, 1), float(b_fp[0]), dtype=np.float32)

    state16 = state_HS.astype(np.float16)
    hats16 = hats.astype(np.float16)

    in_maps, maps = [], []
    for c in range(NC):
        order = orders[c]
        ordp = np.concatenate(
            [order, np.full(geom.NCHUNK - NPER, order[-1], np.int64)])
        degp = np.concatenate(
            [deg_sorted[c], np.zeros(geom.NCHUNK - NPER, np.int64)])

        # state rounds [NR, 128, 2048]
        st_p = np.empty((geom.NR, 128, 2048), np.float16)
        for r in range(geom.NR):
            nodes512 = ordp[512 * r:512 * (r + 1)]
            arr = state16[nodes512]                      # [512, 512]
            st_p[r] = (arr.reshape(512, 4, 128)
                       .transpose(2, 1, 0).reshape(128, 2048))

        # hats segments [NSEG, 128, SEGCOLS]
        hp = np.zeros((geom.NSEG, 128, SEGCOLS), np.float16)
        for b in range(geom.NBLK4):
            blk = geom.blocks[b]
            if blk is None:
                continue
            d, C, ng, W, gplace, ukoff = blk
            ci = 128 * b + np.arange(128)                # chunk index [ng*C]
            nodesb = ordp[ci].reshape(ng, C)
            degb = degp[ci].reshape(ng, C)
            j = np.arange(d)
            e = estart[nodesb][:, :, None] + j[None, None, :]
            valid = j[None, None, :] < degb[:, :, None]
            eidx = np.where(valid, e, 0)
            vals = hats16[eidx]                          # [ng, C, d, 128]
            for g in range(ng):
                s, off = gplace[g]
                hp[s][:, off:off + W] = vals[g].reshape(W, 128).T
        in_maps.append({
            "wcat": wcat, "bfs": bfs_c, "wfp": wfp_c, "bfp": bfp_c,
            "state_p": st_p,
            "hats_p": np.ascontiguousarray(hp),
        })
        maps.append((ordp, degp, estart[ordp]))
    return in_maps, geom, maps


def assemble(results, geom, maps):
    out = np.empty(N_NODES + N_EDGES, dtype=np.float32)
    for c in range(NC):
        ordp, degp, e0p = maps[c]
        uk = np.asarray(results[c]["uk_o"]).reshape(-1)
        u0 = np.asarray(results[c]["u0_o"])              # [128, NBLK4]
        i = np.arange(NPER)
        out[ordp[:NPER]] = u0[i % 128, i // 128]
        # uk: per block, diag output order is [chunk-in-group, group, slot]
        srcs, dsts = [], []
        for b in range(geom.NBLK4):
            blk = geom.blocks[b]
            if blk is None:
                continue
            d, C, ng, W, gplace, ukoff = blk
            ci = 128 * b + np.arange(128)
            degb = degp[ci]
            cc = np.arange(128)                          # chunk-in-block
            g = cc // C
            r = cc % C
            j = np.arange(d)
            pos = ukoff + r[:, None] * (ng * d) + g[:, None] * d + j[None, :]
            valid = j[None, :] < degb[:, None]
            if not valid.any():
                continue
            # edge index needs global estart; recompute cheaply
            srcs.append(pos[valid])
            e0 = e0p[ci]
            dsts.append((e0[:, None] + j[None, :])[valid])
        if srcs:
            sp = np.concatenate(srcs)
            dp = np.concatenate(dsts)
            out[N_NODES + dp] = 1.0 / (1.0 + np.exp(-uk[sp]))
    return out


def kernel(state_HS, hats, seg, W_fs, b_fs, W_fp, b_fp):
    from concourse.bass_utils import run_bass_kernel_spmd
    in_maps, geom, maps = prep_inputs(
        state_HS, hats, seg, W_fs, b_fs, W_fp, b_fp)
    nc = build_nc(geom)
    res = run_bass_kernel_spmd(nc, in_maps, core_ids=list(range(NC)))
    return assemble(res.results, geom, maps)


# revision 5
# speedup vs baseline: 1.2655x; 1.2032x over previous
"""Trainium2 Bass kernel for nn_PolicyNet_78365973283198 (GNN message passing).

Computation (reference):
    tempHS = tanh(state_HS @ W_fs + b_fs)          # [N, 128]
    u0     = tempHS @ W_fp + b_fp                  # [N]
    uk[e]  = <tempHS[seg[e]], hats[e]>             # [E]  (seg sorted)
    out    = sigmoid(concat([u0, uk]))             # [N + E]

Design: data-parallel over nodes on 8 cores (6250 nodes each). Nodes are
sorted by degree (descending); the degree sequence is canonicalized to the
per-rank max across cores so ONE program serves all 8 cores (~4% pad).
Blocks of 128 chunks share a uniform degree d_b (block max).

uk via ALL-PAIRS matmuls: per block, phase A produces thT [d=128, n=128]
(fp16, W hi/lo split for precision). For each group of C chunks
(C = 32/16/8 so W = C*d <= 512), ONE matmul out[c, s] = <th[g*C+c], hats[s]>
against the RAW hats tile (packed [E_DIM, slots] fp16 on host) computes every
needed dot product at 1 PE cycle/slot -- no expansion matmul, no DVE multiply,
no reduce matmul (the old pipeline cost ~2x PE + a DVE pass). The needed
values form a per-group diagonal band; Act/DVE copy the PSUM group tiles into
an SBUF stage and a single 3-dim "diagonal" DMA per block
([(L+d, C), (W, ng), (1, d)] -- partition-crossing stride on dim 0 only,
offset < row length, C <= 42: all hardware-validated) extracts the band
straight to DRAM. u0 rides on the same loaded weights as an N=1 matmul.
Host applies sigmoid to uk during unpack; u0 is sigmoided on device.
"""

import numpy as np

N_NODES = 50000
N_EDGES = 600000
IN_DIM = 512
E_DIM = 128
NC = 8
NPER = N_NODES // NC
SEGCOLS = 8192          # hats segment width (cols) = 2MB fp16 per DMA
HB_BUFS = 4             # hats segment ring depth
STGW = 4352             # uk stage width (f32 cols); caps block degree at 34


def _f16(x):
    return np.ascontiguousarray(x, dtype=np.float16)


def _group_c(d):
    if d <= 16:
        return 32
    if d <= 32:
        return 16
    if d <= 64:
        return 8
    raise AssertionError(f"degree {d} > 64 unsupported")


class Geom:
    """Canonical (core-independent) program geometry."""

    def __init__(self, D):
        # D: canonical per-chunk degrees, len NPER, sorted descending
        NBLK = -(-NPER // 128)
        self.NR = -(-NBLK // 4)
        self.NBLK4 = 4 * self.NR
        self.NCHUNK = 128 * self.NBLK4
        Dp = np.zeros(self.NCHUNK, np.int64)
        Dp[:NPER] = D
        self.d_b = [int(Dp[128 * b]) for b in range(self.NBLK4)]
        assert 128 * max(self.d_b) <= STGW, f"max block degree {max(self.d_b)}"

        self.blocks = []        # per block: None or (d, C, ng, W, [(seg, off)]*ng, ukoff)
        seg_i, cur, ukoff = 0, 0, 0
        for b in range(self.NBLK4):
            d = self.d_b[b]
            if d == 0:
                self.blocks.append(None)
                continue
            C = _group_c(d)
            ng = 128 // C
            W = C * d
            gplace = []
            for g in range(ng):
                if cur + W > SEGCOLS:
                    seg_i += 1
                    cur = 0
                gplace.append((seg_i, cur))
                cur += W
            self.blocks.append((d, C, ng, W, gplace, ukoff))
            ukoff += 128 * d
        self.NSEG = seg_i + 1
        self.SUK = max(ukoff, 1)


def emit(nc, t, geom):
    import concourse.tile as tile
    from concourse import mybir
    from concourse.ap import AP

    fp16 = mybir.dt.float16
    f32 = mybir.dt.float32
    Act = mybir.ActivationFunctionType

    wcat_d, bfs_d, wfp_d, bfp_d = t["wcat"], t["bfs"], t["wfp"], t["bfp"]
    state_d, hats_d = t["state_p"], t["hats_p"]
    uk_d, u0_d = t["uk_o"], t["u0_o"]
    NR, NSEG, NBLK4 = geom.NR, geom.NSEG, geom.NBLK4

    with tile.TileContext(nc) as tc:
        with (
            tc.tile_pool(name="const", bufs=1) as cpool,
            tc.tile_pool(name="perst", bufs=1) as ppool,
            tc.tile_pool(name="st", bufs=3) as stpool,
            tc.tile_pool(name="th", bufs=3) as thpool,
            tc.tile_pool(name="hat", bufs=HB_BUFS) as hpool,
            tc.tile_pool(name="stg", bufs=4) as sgpool,
            tc.tile_pool(name="psA", bufs=2, space="PSUM") as psA,
            tc.tile_pool(name="psK", bufs=5, space="PSUM") as psK,
            tc.tile_pool(name="psU", bufs=1, space="PSUM") as psU,
        ):
            wcat = cpool.tile([128, 8 * 128], fp16, tag="wcat")
            nc.sync.dma_start(wcat[:], wcat_d[:])
            bfs = cpool.tile([128, 1], fp16, tag="bfs")
            nc.sync.dma_start(bfs[:], bfs_d[:])
            wfp = cpool.tile([128, 1], fp16, tag="wfp")
            nc.sync.dma_start(wfp[:], wfp_d[:])
            bfp = cpool.tile([128, 1], f32, tag="bfp")
            nc.sync.dma_start(bfp[:], bfp_d[:])

            u0acc = psU.tile([128, NBLK4], f32, tag="u0acc")

            # hats segment ring with explicit prefetch emission
            hseg = {}
            next_seg = [0]

            def fetch_seg():
                s = next_seg[0]
                if s >= NSEG:
                    return
                tile_ = hpool.tile([128, SEGCOLS], fp16, tag="hseg")
                # SWDGE: keeps bulk loads off the sync/scalar queues so the
                # diag DMAs and Act compute never stall behind a buffer wait
                nc.gpsimd.dma_start(tile_[:], hats_d[s])
                hseg[s] = tile_
                next_seg[0] += 1

            for _ in range(min(HB_BUFS - 1, NSEG)):
                fetch_seg()

            st_tiles = {}
            next_st = [0]

            def fetch_st():
                r = next_st[0]
                if r >= NR:
                    return
                tile_ = stpool.tile([128, 2048], fp16, tag="st")
                nc.sync.dma_start(tile_[:], state_d[r])
                st_tiles[r] = tile_
                next_st[0] += 1

            fetch_st()
            fetch_st()

            cp_i = [0]          # copy engine round robin

            def copy(out_ap, in_ap):
                # weight DVE slightly more than Act (Act also does tanh+DMA)
                k = cp_i[0] % 5
                cp_i[0] += 1
                if k in (0, 1, 3):
                    nc.vector.tensor_scalar_add(out=out_ap, in0=in_ap, scalar1=0.0)
                else:
                    nc.scalar.activation(out_ap, in_ap, Act.Copy)

            for r in range(NR):
                stb = st_tiles.pop(r)
                fetch_st()
                tp = psA.tile([128, 512], f32, tag="tp")
                for i in range(8):          # (hi/lo, c4) combos
                    c4 = i % 4
                    nc.tensor.matmul(
                        tp[:],
                        lhsT=wcat[:, i * 128:(i + 1) * 128],
                        rhs=stb[:, c4 * 512:(c4 + 1) * 512],
                        start=(i == 0), stop=(i == 7),
                    )
                thT4 = thpool.tile([128, 512], fp16, tag="thT4")
                nc.scalar.activation(thT4[:], tp[:], Act.Tanh, bias=bfs[:, 0:1])

                for b4 in range(4):
                    b = 4 * r + b4
                    thT = thT4[:, b4 * 128:(b4 + 1) * 128]
                    nc.tensor.matmul(u0acc[:, b:b + 1], lhsT=thT, rhs=wfp[:],
                                     start=True, stop=True)
                    blk = geom.blocks[b]
                    if blk is None:
                        continue
                    d, C, ng, W, gplace, ukoff = blk
                    LB = ng * W     # = 128*d
                    stage = sgpool.tile([128, STGW], f32, tag="stage")
                    for g in range(ng):
                        s, off = gplace[g]
                        while next_seg[0] <= s + (HB_BUFS - 2):
                            if next_seg[0] >= NSEG:
                                break
                            fetch_seg()
                        hs = hseg[s]
                        pk = psK.tile([128, 512], f32, tag="pk")
                        nc.tensor.matmul(
                            pk[0:C, 0:W],
                            lhsT=thT[:, g * C:(g + 1) * C],
                            rhs=hs[:, off:off + W],
                            start=True, stop=True)
                        copy(stage[0:C, g * W:(g + 1) * W], pk[0:C, 0:W])
                    sap = stage[:]
                    diag = AP(sap.tensor, sap.offset,
                              [(STGW + d, C), (W, ng), (1, d)])
                    nc.sync.dma_start(uk_d[0:1, ukoff:ukoff + 128 * d], diag)

            u0sb = ppool.tile([128, NBLK4], f32, tag="u0sb")
            nc.scalar.activation(u0sb[:], u0acc[:], Act.Sigmoid, bias=bfp[:, 0:1])
            nc.sync.dma_start(u0_d[:], u0sb[:])
    return []


def build_nc(geom):
    import concourse.bass as bass
    from concourse import mybir

    fp16 = mybir.dt.float16
    f32 = mybir.dt.float32

    nc = bass.Bass("TRN2", target_bir_lowering=False, debug=False)
    t = {
        "wcat": nc.dram_tensor("wcat", [128, 8 * 128], fp16,
                               kind="ExternalInput")[:],
        "bfs": nc.dram_tensor("bfs", [128, 1], fp16, kind="ExternalInput")[:],
        "wfp": nc.dram_tensor("wfp", [128, 1], fp16, kind="ExternalInput")[:],
        "bfp": nc.dram_tensor("bfp", [128, 1], f32, kind="ExternalInput")[:],
        "state_p": nc.dram_tensor("state_p", [geom.NR, 128, 2048], fp16,
                                  kind="ExternalInput"),
        "hats_p": nc.dram_tensor("hats_p", [geom.NSEG, 128, SEGCOLS], fp16,
                                 kind="ExternalInput"),
        "uk_o": nc.dram_tensor("uk_o", [1, geom.SUK], f32,
                               kind="ExternalOutput")[:],
        "u0_o": nc.dram_tensor("u0_o", [128, geom.NBLK4], f32,
                               kind="ExternalOutput")[:],
    }
    emit(nc, t, geom)
    split_multi_waits(nc)
    return nc


def split_multi_waits(nc):
    """This env's walrus encodes at most one sem wait per instruction; hoist
    extras onto standalone EventSemaphore insts immediately before."""
    import concourse.mybir as mybir
    n = 0
    for fn in nc.m.functions:
        for bb in fn.blocks:
            insts = list(bb.instructions)
            if not any(i.sync_info and len(i.sync_info.on_wait) > 1 for i in insts):
                continue
            out = []
            for inst in insts:
                si = inst.sync_info
                if si is not None and len(si.on_wait) > 1:
                    waits = list(si.on_wait)
                    for w in waits[:-1]:
                        n += 1
                        out.append(mybir.InstEventSemaphore(
                            name=f"splitw_{n}_{inst.name}",
                            engine=inst.engine, ins=[], outs=[],
                            sync_info=mybir.SyncInfo(on_wait=[w], on_update=[]),
                        ))
                    inst.sync_info = mybir.SyncInfo(
                        on_wait=[waits[-1]], on_update=list(si.on_update))
                out.append(inst)
            bb.instructions = out
    return n


def prep_inputs(state_HS, hats, seg, W_fs, b_fs, W_fp, b_fp):
    """Shard + pack. Returns (in_maps, geom, maps)."""
    state_HS = np.asarray(state_HS, dtype=np.float32)
    hats = np.asarray(hats, dtype=np.float32)
    seg = np.asarray(seg, dtype=np.int32)
    W_fs = np.asarray(W_fs, dtype=np.float32)
    b_fs = np.asarray(b_fs, dtype=np.float32)
    W_fp = np.asarray(W_fp, dtype=np.float32)
    b_fp = np.asarray(b_fp, dtype=np.float32)

    deg = np.bincount(seg, minlength=N_NODES).astype(np.int64)
    estart = np.concatenate([[0], np.cumsum(deg)[:-1]])

    orders = []
    deg_sorted = np.empty((NC, NPER), np.int64)
    for c in range(NC):
        nodes = np.arange(c * NPER, (c + 1) * NPER)
        o = np.lexsort((nodes, -deg[nodes]))
        orders.append(nodes[o])
        deg_sorted[c] = deg[nodes[o]]
    D = deg_sorted.max(axis=0)
    geom = Geom(D)

    # constants
    w_hi = W_fs.astype(np.float16)
    w_lo = (W_fs.astype(np.float64) - w_hi.astype(np.float64)).astype(np.float16)
    wcat = np.empty((128, 8, 128), dtype=np.float16)
    for hl, w in enumerate((w_hi, w_lo)):
        for c4 in range(4):
            wcat[:, hl * 4 + c4, :] = w[c4 * 128:(c4 + 1) * 128, :]
    wcat = wcat.reshape(128, 8 * 128)
    bfs_c = _f16(b_fs.reshape(128, 1))
    wfp_c = _f16(W_fp.reshape(128, 1))
    bfp_c = np.full((128, 1), float(b_fp[0]), dtype=np.float32)

    state16 = state_HS.astype(np.float16)
    hats16 = hats.astype(np.float16)

    in_maps, maps = [], []
    for c in range(NC):
        order = orders[c]
        ordp = np.concatenate(
            [order, np.full(geom.NCHUNK - NPER, order[-1], np.int64)])
        degp = np.concatenate(
            [deg_sorted[c], np.zeros(geom.NCHUNK - NPER, np.int64)])

        # state rounds [NR, 128, 2048]
        st_p = np.empty((geom.NR, 128, 2048), np.float16)
        for r in range(geom.NR):
            nodes512 = ordp[512 * r:512 * (r + 1)]
            arr = state16[nodes512]                      # [512, 512]
            st_p[r] = (arr.reshape(512, 4, 128)
                       .transpose(2, 1, 0).reshape(128, 2048))

        # hats segments [NSEG, 128, SEGCOLS]
        hp = np.zeros((geom.NSEG, 128, SEGCOLS), np.float16)
        for b in range(geom.NBLK4):
            blk = geom.blocks[b]
            if blk is None:
                continue
            d, C, ng, W, gplace, ukoff = blk
            ci = 128 * b + np.arange(128)                # chunk index [ng*C]
            nodesb = ordp[ci].reshape(ng, C)
            degb = degp[ci].reshape(ng, C)
            j = np.arange(d)
            e = estart[nodesb][:, :, None] + j[None, None, :]
            valid = j[None, None, :] < degb[:, :, None]
            eidx = np.where(valid, e, 0)
            vals = hats16[eidx]                          # [ng, C, d, 128]
            for g in range(ng):
                s, off = gplace[g]
                hp[s][:, off:off + W] = vals[g].reshape(W, 128).T
        in_maps.append({
            "wcat": wcat, "bfs": bfs_c, "wfp": wfp_c, "bfp": bfp_c,
            "state_p": st_p,
            "hats_p": np.ascontiguousarray(hp),
        })
        maps.append((ordp, degp, estart[ordp]))
    return in_maps, geom, maps


def assemble(results, geom, maps):
    out = np.empty(N_NODES + N_EDGES, dtype=np.float32)
    for c in range(NC):
        ordp, degp, e0p = maps[c]
        uk = np.asarray(results[c]["uk_o"]).reshape(-1)
        u0 = np.asarray(results[c]["u0_o"])              # [128, NBLK4]
        i = np.arange(NPER)
        out[ordp[:NPER]] = u0[i % 128, i // 128]
        # uk: per block, diag output order is [chunk-in-group, group, slot]
        srcs, dsts = [], []
        for b in range(geom.NBLK4):
            blk = geom.blocks[b]
            if blk is None:
                continue
            d, C, ng, W, gplace, ukoff = blk
            ci = 128 * b + np.arange(128)
            degb = degp[ci]
            cc = np.arange(128)                          # chunk-in-block
            g = cc // C
            r = cc % C
            j = np.arange(d)
            pos = ukoff + r[:, None] * (ng * d) + g[:, None] * d + j[None, :]
            valid = j[None, :] < degb[:, None]
            if not valid.any():
                continue
            # edge index needs global estart; recompute cheaply
            srcs.append(pos[valid])
            e0 = e0p[ci]
            dsts.append((e0[:, None] + j[None, :])[valid])
        if srcs:
            sp = np.concatenate(srcs)
            dp = np.concatenate(dsts)
            out[N_NODES + dp] = 1.0 / (1.0 + np.exp(-uk[sp]))
    return out


def kernel(state_HS, hats, seg, W_fs, b_fs, W_fp, b_fp):
    from concourse.bass_utils import run_bass_kernel_spmd
    in_maps, geom, maps = prep_inputs(
        state_HS, hats, seg, W_fs, b_fs, W_fp, b_fp)
    nc = build_nc(geom)
    res = run_bass_kernel_spmd(nc, in_maps, core_ids=list(range(NC)))
    return assemble(res.results, geom, maps)


# revision 6
# speedup vs baseline: 1.3329x; 1.0532x over previous
"""Trainium2 Bass kernel for nn_PolicyNet_78365973283198 (GNN message passing).

Computation (reference):
    tempHS = tanh(state_HS @ W_fs + b_fs)          # [N, 128]
    u0     = tempHS @ W_fp + b_fp                  # [N]
    uk[e]  = <tempHS[seg[e]], hats[e]>             # [E]  (seg sorted)
    out    = sigmoid(concat([u0, uk]))             # [N + E]

Design: data-parallel over nodes on 8 cores (6250 nodes each). Nodes are
sorted by degree (descending); the degree sequence is canonicalized to the
per-rank max across cores so ONE program serves all 8 cores (~4% pad).
Blocks of 128 chunks share a uniform degree d_b (block max).

uk via ALL-PAIRS matmuls: per block, phase A produces thT [d=128, n=128]
(fp16, W hi/lo split for precision). For each group of C chunks
(C = 32/16/8 so W = C*d <= 512), ONE matmul out[c, s] = <th[g*C+c], hats[s]>
against the RAW hats tile (packed [E_DIM, slots] fp16 on host) computes every
needed dot product at 1 PE cycle/slot -- no expansion matmul, no DVE multiply,
no reduce matmul (the old pipeline cost ~2x PE + a DVE pass). The needed
values form a per-group diagonal band; Act/DVE copy the PSUM group tiles into
an SBUF stage and a single 3-dim "diagonal" DMA per block
([(L+d, C), (W, ng), (1, d)] -- partition-crossing stride on dim 0 only,
offset < row length, C <= 42: all hardware-validated) extracts the band
straight to DRAM. u0 rides on the same loaded weights as an N=1 matmul.
Host applies sigmoid to uk during unpack; u0 is sigmoided on device.
"""

import numpy as np

N_NODES = 50000
N_EDGES = 600000
IN_DIM = 512
E_DIM = 128
NC = 8
NPER = N_NODES // NC
SEGCOLS = 8192          # hats segment width (cols) = 2MB fp16 per DMA
HB_BUFS = 4             # hats segment ring depth
STGW = 4352             # uk stage width (f32 cols); caps block degree at 34


def _f16(x):
    return np.ascontiguousarray(x, dtype=np.float16)


def _group_c(d):
    if d <= 16:
        return 32
    if d <= 32:
        return 16
    if d <= 64:
        return 8
    raise AssertionError(f"degree {d} > 64 unsupported")


class Geom:
    """Canonical (core-independent) program geometry."""

    def __init__(self, D):
        # D: canonical per-chunk degrees, len NPER, sorted descending
        NBLK = -(-NPER // 128)
        self.NR = -(-NBLK // 4)
        self.NBLK4 = 4 * self.NR
        self.NCHUNK = 128 * self.NBLK4
        Dp = np.zeros(self.NCHUNK, np.int64)
        Dp[:NPER] = D
        self.d_b = [int(Dp[128 * b]) for b in range(self.NBLK4)]
        assert 128 * max(self.d_b) <= STGW, f"max block degree {max(self.d_b)}"

        self.blocks = []        # per block: None or (d, C, ng, W, [(seg, off)]*ng, ukoff)
        seg_i, cur, ukoff = 0, 0, 0
        for b in range(self.NBLK4):
            d = self.d_b[b]
            if d == 0:
                self.blocks.append(None)
                continue
            C = _group_c(d)
            ng = 128 // C
            W = C * d
            gplace = []
            for g in range(ng):
                if cur + W > SEGCOLS:
                    seg_i += 1
                    cur = 0
                gplace.append((seg_i, cur))
                cur += W
            self.blocks.append((d, C, ng, W, gplace, ukoff))
            ukoff += 128 * d
        self.NSEG = seg_i + 1
        self.SUK = max(ukoff, 1)


def emit(nc, t, geom):
    import concourse.tile as tile
    from concourse import mybir
    from concourse.ap import AP

    fp16 = mybir.dt.float16
    f32 = mybir.dt.float32
    Act = mybir.ActivationFunctionType

    wcat_d, bfs_d, wfp_d, bfp_d = t["wcat"], t["bfs"], t["wfp"], t["bfp"]
    state_d, hats_d = t["state_p"], t["hats_p"]
    uk_d, u0_d = t["uk_o"], t["u0_o"]
    NR, NSEG, NBLK4 = geom.NR, geom.NSEG, geom.NBLK4

    with tile.TileContext(nc) as tc:
        with (
            tc.tile_pool(name="const", bufs=1) as cpool,
            tc.tile_pool(name="perst", bufs=1) as ppool,
            tc.tile_pool(name="st", bufs=3) as stpool,
            tc.tile_pool(name="th", bufs=3) as thpool,
            tc.tile_pool(name="hat", bufs=HB_BUFS) as hpool,
            tc.tile_pool(name="stg", bufs=4) as sgpool,
            tc.tile_pool(name="psA", bufs=2, space="PSUM") as psA,
            tc.tile_pool(name="psK", bufs=5, space="PSUM") as psK,
            tc.tile_pool(name="psU", bufs=1, space="PSUM") as psU,
        ):
            wcat = cpool.tile([128, 8 * 128], fp16, tag="wcat")
            nc.sync.dma_start(wcat[:], wcat_d[:])
            bfs = cpool.tile([128, 1], fp16, tag="bfs")
            nc.sync.dma_start(bfs[:], bfs_d[:])
            wfp = cpool.tile([128, 1], fp16, tag="wfp")
            nc.sync.dma_start(wfp[:], wfp_d[:])
            bfp = cpool.tile([128, 1], f32, tag="bfp")
            nc.sync.dma_start(bfp[:], bfp_d[:])

            u0acc = psU.tile([128, NBLK4], f32, tag="u0acc")

            # hats segment ring with explicit prefetch emission
            hseg = {}
            next_seg = [0]

            def fetch_seg():
                s = next_seg[0]
                if s >= NSEG:
                    return
                tile_ = hpool.tile([128, SEGCOLS], fp16, tag="hseg")
                # SWDGE: keeps bulk loads off the sync/scalar queues so the
                # diag DMAs and Act compute never stall behind a buffer wait
                nc.gpsimd.dma_start(tile_[:], hats_d[s])
                hseg[s] = tile_
                next_seg[0] += 1

            st_tiles = {}
            next_st = [0]

            def fetch_st():
                r = next_st[0]
                if r >= NR:
                    return
                tile_ = stpool.tile([128, 2048], fp16, tag="st")
                nc.sync.dma_start(tile_[:], state_d[r])
                st_tiles[r] = tile_
                next_st[0] += 1

            # state first: the first phase-A round must not queue behind the
            # 6MB hats prefetch burst (SDMA fair-shares rings at packet level)
            fetch_st()
            fetch_st()
            for _ in range(min(HB_BUFS - 1, NSEG)):
                fetch_seg()

            cp_i = [0]          # copy engine round robin

            def copy(out_ap, in_ap):
                # weight DVE slightly more than Act (Act also does tanh+DMA)
                k = cp_i[0] % 5
                cp_i[0] += 1
                if k in (0, 1, 3):
                    nc.vector.tensor_scalar_add(out=out_ap, in0=in_ap, scalar1=0.0)
                else:
                    nc.scalar.activation(out_ap, in_ap, Act.Copy)

            for r in range(NR):
                stb = st_tiles.pop(r)
                fetch_st()
                tp = psA.tile([128, 512], f32, tag="tp")
                for i in range(8):          # (hi/lo, c4) combos
                    c4 = i % 4
                    nc.tensor.matmul(
                        tp[:],
                        lhsT=wcat[:, i * 128:(i + 1) * 128],
                        rhs=stb[:, c4 * 512:(c4 + 1) * 512],
                        start=(i == 0), stop=(i == 7),
                    )
                thT4 = thpool.tile([128, 512], fp16, tag="thT4")
                nc.scalar.activation(thT4[:], tp[:], Act.Tanh, bias=bfs[:, 0:1])

                for b4 in range(4):
                    b = 4 * r + b4
                    thT = thT4[:, b4 * 128:(b4 + 1) * 128]
                    nc.tensor.matmul(u0acc[:, b:b + 1], lhsT=thT, rhs=wfp[:],
                                     start=True, stop=True)
                    blk = geom.blocks[b]
                    if blk is None:
                        continue
                    d, C, ng, W, gplace, ukoff = blk
                    LB = ng * W     # = 128*d
                    stage = sgpool.tile([128, STGW], f32, tag="stage")
                    for g in range(ng):
                        s, off = gplace[g]
                        while next_seg[0] <= s + (HB_BUFS - 2):
                            if next_seg[0] >= NSEG:
                                break
                            fetch_seg()
                        hs = hseg[s]
                        pk = psK.tile([128, 512], f32, tag="pk")
                        nc.tensor.matmul(
                            pk[0:C, 0:W],
                            lhsT=thT[:, g * C:(g + 1) * C],
                            rhs=hs[:, off:off + W],
                            start=True, stop=True)
                        copy(stage[0:C, g * W:(g + 1) * W], pk[0:C, 0:W])
                    sap = stage[:]
                    diag = AP(sap.tensor, sap.offset,
                              [(STGW + d, C), (W, ng), (1, d)])
                    nc.sync.dma_start(uk_d[0:1, ukoff:ukoff + 128 * d], diag)

            u0sb = ppool.tile([128, NBLK4], f32, tag="u0sb")
            nc.scalar.activation(u0sb[:], u0acc[:], Act.Sigmoid, bias=bfp[:, 0:1])
            nc.sync.dma_start(u0_d[:], u0sb[:])
    return []


def build_nc(geom):
    import concourse.bass as bass
    from concourse import mybir

    fp16 = mybir.dt.float16
    f32 = mybir.dt.float32

    nc = bass.Bass("TRN2", target_bir_lowering=False, debug=False)
    t = {
        "wcat": nc.dram_tensor("wcat", [128, 8 * 128], fp16,
                               kind="ExternalInput")[:],
        "bfs": nc.dram_tensor("bfs", [128, 1], fp16, kind="ExternalInput")[:],
        "wfp": nc.dram_tensor("wfp", [128, 1], fp16, kind="ExternalInput")[:],
        "bfp": nc.dram_tensor("bfp", [128, 1], f32, kind="ExternalInput")[:],
        "state_p": nc.dram_tensor("state_p", [geom.NR, 128, 2048], fp16,
                                  kind="ExternalInput"),
        "hats_p": nc.dram_tensor("hats_p", [geom.NSEG, 128, SEGCOLS], fp16,
                                 kind="ExternalInput"),
        "uk_o": nc.dram_tensor("uk_o", [1, geom.SUK], f32,
                               kind="ExternalOutput")[:],
        "u0_o": nc.dram_tensor("u0_o", [128, geom.NBLK4], f32,
                               kind="ExternalOutput")[:],
    }
    emit(nc, t, geom)
    split_multi_waits(nc)
    return nc


def split_multi_waits(nc):
    """This env's walrus encodes at most one sem wait per instruction; hoist
    extras onto standalone EventSemaphore insts immediately before."""
    import concourse.mybir as mybir
    n = 0
    for fn in nc.m.functions:
        for bb in fn.blocks:
            insts = list(bb.instructions)
            if not any(i.sync_info and len(i.sync_info.on_wait) > 1 for i in insts):
                continue
            out = []
            for inst in insts:
                si = inst.sync_info
                if si is not None and len(si.on_wait) > 1:
                    waits = list(si.on_wait)
                    for w in waits[:-1]:
                        n += 1
                        out.append(mybir.InstEventSemaphore(
                            name=f"splitw_{n}_{inst.name}",
                            engine=inst.engine, ins=[], outs=[],
                            sync_info=mybir.SyncInfo(on_wait=[w], on_update=[]),
                        ))
                    inst.sync_info = mybir.SyncInfo(
                        on_wait=[waits[-1]], on_update=list(si.on_update))
                out.append(inst)
            bb.instructions = out
    return n


def prep_inputs(state_HS, hats, seg, W_fs, b_fs, W_fp, b_fp):
    """Shard + pack. Returns (in_maps, geom, maps)."""
    state_HS = np.asarray(state_HS, dtype=np.float32)
    hats = np.asarray(hats, dtype=np.float32)
    seg = np.asarray(seg, dtype=np.int32)
    W_fs = np.asarray(W_fs, dtype=np.float32)
    b_fs = np.asarray(b_fs, dtype=np.float32)
    W_fp = np.asarray(W_fp, dtype=np.float32)
    b_fp = np.asarray(b_fp, dtype=np.float32)

    deg = np.bincount(seg, minlength=N_NODES).astype(np.int64)
    estart = np.concatenate([[0], np.cumsum(deg)[:-1]])

    orders = []
    deg_sorted = np.empty((NC, NPER), np.int64)
    for c in range(NC):
        nodes = np.arange(c * NPER, (c + 1) * NPER)
        o = np.lexsort((nodes, -deg[nodes]))
        orders.append(nodes[o])
        deg_sorted[c] = deg[nodes[o]]
    D = deg_sorted.max(axis=0)
    geom = Geom(D)

    # constants
    w_hi = W_fs.astype(np.float16)
    w_lo = (W_fs.astype(np.float64) - w_hi.astype(np.float64)).astype(np.float16)
    wcat = np.empty((128, 8, 128), dtype=np.float16)
    for hl, w in enumerate((w_hi, w_lo)):
        for c4 in range(4):
            wcat[:, hl * 4 + c4, :] = w[c4 * 128:(c4 + 1) * 128, :]
    wcat = wcat.reshape(128, 8 * 128)
    bfs_c = _f16(b_fs.reshape(128, 1))
    wfp_c = _f16(W_fp.reshape(128, 1))
    bfp_c = np.full((128, 1), float(b_fp[0]), dtype=np.float32)

    state16 = state_HS.astype(np.float16)
    hats16 = hats.astype(np.float16)

    in_maps, maps = [], []
    for c in range(NC):
        order = orders[c]
        ordp = np.concatenate(
            [order, np.full(geom.NCHUNK - NPER, order[-1], np.int64)])
        degp = np.concatenate(
            [deg_sorted[c], np.zeros(geom.NCHUNK - NPER, np.int64)])

        # state rounds [NR, 128, 2048]
        st_p = np.empty((geom.NR, 128, 2048), np.float16)
        for r in range(geom.NR):
            nodes512 = ordp[512 * r:512 * (r + 1)]
            arr = state16[nodes512]                      # [512, 512]
            st_p[r] = (arr.reshape(512, 4, 128)
                       .transpose(2, 1, 0).reshape(128, 2048))

        # hats segments [NSEG, 128, SEGCOLS]
        hp = np.zeros((geom.NSEG, 128, SEGCOLS), np.float16)
        for b in range(geom.NBLK4):
            blk = geom.blocks[b]
            if blk is None:
                continue
            d, C, ng, W, gplace, ukoff = blk
            ci = 128 * b + np.arange(128)                # chunk index [ng*C]
            nodesb = ordp[ci].reshape(ng, C)
            degb = degp[ci].reshape(ng, C)
            j = np.arange(d)
            e = estart[nodesb][:, :, None] + j[None, None, :]
            valid = j[None, None, :] < degb[:, :, None]
            eidx = np.where(valid, e, 0)
            vals = hats16[eidx]                          # [ng, C, d, 128]
            for g in range(ng):
                s, off = gplace[g]
                hp[s][:, off:off + W] = vals[g].reshape(W, 128).T
        in_maps.append({
            "wcat": wcat, "bfs": bfs_c, "wfp": wfp_c, "bfp": bfp_c,
            "state_p": st_p,
            "hats_p": np.ascontiguousarray(hp),
        })
        maps.append((ordp, degp, estart[ordp]))
    return in_maps, geom, maps


def assemble(results, geom, maps):
    out = np.empty(N_NODES + N_EDGES, dtype=np.float32)
    for c in range(NC):
        ordp, degp, e0p = maps[c]
        uk = np.asarray(results[c]["uk_o"]).reshape(-1)
        u0 = np.asarray(results[c]["u0_o"])              # [128, NBLK4]
        i = np.arange(NPER)
        out[ordp[:NPER]] = u0[i % 128, i // 128]
        # uk: per block, diag output order is [chunk-in-group, group, slot]
        srcs, dsts = [], []
        for b in range(geom.NBLK4):
            blk = geom.blocks[b]
            if blk is None:
                continue
            d, C, ng, W, gplace, ukoff = blk
            ci = 128 * b + np.arange(128)
            degb = degp[ci]
            cc = np.arange(128)                          # chunk-in-block
            g = cc // C
            r = cc % C
            j = np.arange(d)
            pos = ukoff + r[:, None] * (ng * d) + g[:, None] * d + j[None, :]
            valid = j[None, :] < degb[:, None]
            if not valid.any():
                continue
            # edge index needs global estart; recompute cheaply
            srcs.append(pos[valid])
            e0 = e0p[ci]
            dsts.append((e0[:, None] + j[None, :])[valid])
        if srcs:
            sp = np.concatenate(srcs)
            dp = np.concatenate(dsts)
            out[N_NODES + dp] = 1.0 / (1.0 + np.exp(-uk[sp]))
    return out


def kernel(state_HS, hats, seg, W_fs, b_fs, W_fp, b_fp):
    from concourse.bass_utils import run_bass_kernel_spmd
    in_maps, geom, maps = prep_inputs(
        state_HS, hats, seg, W_fs, b_fs, W_fp, b_fp)
    nc = build_nc(geom)
    res = run_bass_kernel_spmd(nc, in_maps, core_ids=list(range(NC)))
    return assemble(res.results, geom, maps)


# revision 7
# speedup vs baseline: 1.4509x; 1.0886x over previous
"""Trainium2 Bass kernel for nn_PolicyNet_78365973283198 (GNN message passing).

Computation (reference):
    tempHS = tanh(state_HS @ W_fs + b_fs)          # [N, 128]
    u0     = tempHS @ W_fp + b_fp                  # [N]
    uk[e]  = <tempHS[seg[e]], hats[e]>             # [E]  (seg sorted)
    out    = sigmoid(concat([u0, uk]))             # [N + E]

Design: data-parallel over nodes on 8 cores (6250 nodes each). Nodes are
sorted by degree (descending); the degree sequence is canonicalized to the
per-rank max across cores so ONE program serves all 8 cores (~4% pad).
Blocks of 128 chunks share a uniform degree d_b (block max).

uk via ALL-PAIRS matmuls: per block, phase A produces thT [d=128, n=128]
(fp16, W hi/lo split for precision). For each group of C chunks
(C = 32/16/8 so W = C*d <= 512), ONE matmul out[c, s] = <th[g*C+c], hats[s]>
against the RAW hats tile (packed [E_DIM, slots] fp16 on host) computes every
needed dot product at 1 PE cycle/slot -- no expansion matmul, no DVE multiply,
no reduce matmul (the old pipeline cost ~2x PE + a DVE pass). The needed
values form a per-group diagonal band; Act/DVE copy the PSUM group tiles into
an SBUF stage and a single 3-dim "diagonal" DMA per block
([(L+d, C), (W, ng), (1, d)] -- partition-crossing stride on dim 0 only,
offset < row length, C <= 42: all hardware-validated) extracts the band
straight to DRAM. u0 rides on the same loaded weights as an N=1 matmul.
Host applies sigmoid to uk during unpack; u0 is sigmoided on device.
"""

import numpy as np

N_NODES = 50000
N_EDGES = 600000
IN_DIM = 512
E_DIM = 128
NC = 8
NPER = N_NODES // NC
SEGCOLS = 8192          # hats segment width (cols) = 2MB fp16 per DMA
HB_BUFS = 4             # hats segment ring depth
STGW = 4352             # uk stage width (f32 cols); caps block degree at 34


def _f16(x):
    return np.ascontiguousarray(x, dtype=np.float16)


def _group_c(d):
    if d <= 16:
        return 32
    if d <= 32:
        return 16
    if d <= 64:
        return 8
    raise AssertionError(f"degree {d} > 64 unsupported")


class Geom:
    """Canonical (core-independent) program geometry."""

    def __init__(self, D):
        # D: canonical per-chunk degrees, len NPER, sorted descending
        NBLK = -(-NPER // 128)
        self.NR = -(-NBLK // 4)
        self.NBLK4 = 4 * self.NR
        self.NCHUNK = 128 * self.NBLK4
        Dp = np.zeros(self.NCHUNK, np.int64)
        Dp[:NPER] = D
        self.d_b = [int(Dp[128 * b]) for b in range(self.NBLK4)]
        assert 128 * max(self.d_b) <= STGW, f"max block degree {max(self.d_b)}"

        self.blocks = []        # per block: None or (d, C, ng, W, [(seg, off)]*ng, ukoff)
        seg_i, cur, ukoff = 0, 0, 0
        for b in range(self.NBLK4):
            d = self.d_b[b]
            if d == 0:
                self.blocks.append(None)
                continue
            C = _group_c(d)
            ng = 128 // C
            W = C * d
            gplace = []
            for g in range(ng):
                if cur + W > SEGCOLS:
                    seg_i += 1
                    cur = 0
                gplace.append((seg_i, cur))
                cur += W
            self.blocks.append((d, C, ng, W, gplace, ukoff))
            ukoff += 128 * d
        self.NSEG = seg_i + 1
        self.SUK = max(ukoff, 1)


def emit(nc, t, geom):
    import concourse.tile as tile
    from concourse import mybir
    from concourse.ap import AP

    fp16 = mybir.dt.float16
    f32 = mybir.dt.float32
    Act = mybir.ActivationFunctionType

    wcat_d, bfs_d, wfp_d, bfp_d = t["wcat"], t["bfs"], t["wfp"], t["bfp"]
    state_d, hats_d = t["state_p"], t["hats_p"]
    uk_d, u0_d = t["uk_o"], t["u0_o"]
    NR, NSEG, NBLK4 = geom.NR, geom.NSEG, geom.NBLK4

    with tile.TileContext(nc) as tc:
        with (
            tc.tile_pool(name="const", bufs=1) as cpool,
            tc.tile_pool(name="perst", bufs=1) as ppool,
            tc.tile_pool(name="st", bufs=4) as stpool,
            tc.tile_pool(name="th", bufs=3) as thpool,
            tc.tile_pool(name="hat", bufs=HB_BUFS) as hpool,
            tc.tile_pool(name="stg", bufs=4) as sgpool,
            tc.tile_pool(name="psA", bufs=2, space="PSUM") as psA,
            tc.tile_pool(name="psK", bufs=5, space="PSUM") as psK,
            tc.tile_pool(name="psU", bufs=1, space="PSUM") as psU,
        ):
            wcat = cpool.tile([128, 8 * 128], fp16, tag="wcat")
            nc.sync.dma_start(wcat[:], wcat_d[:])
            bfs = cpool.tile([128, 1], fp16, tag="bfs")
            nc.sync.dma_start(bfs[:], bfs_d[:])
            wfp = cpool.tile([128, 1], fp16, tag="wfp")
            nc.sync.dma_start(wfp[:], wfp_d[:])
            bfp = cpool.tile([128, 1], f32, tag="bfp")
            nc.sync.dma_start(bfp[:], bfp_d[:])

            u0acc = psU.tile([128, NBLK4], f32, tag="u0acc")

            # hats segment ring with explicit prefetch emission
            hseg = {}
            next_seg = [0]

            def fetch_seg():
                s = next_seg[0]
                if s >= NSEG:
                    return
                tile_ = hpool.tile([128, SEGCOLS], fp16, tag="hseg")
                # SWDGE: keeps bulk loads off the sync/scalar queues so the
                # diag DMAs and Act compute never stall behind a buffer wait
                nc.gpsimd.dma_start(tile_[:], hats_d[s])
                hseg[s] = tile_
                next_seg[0] += 1

            st_tiles = {}
            next_st = [0]

            def fetch_st():
                r = next_st[0]
                if r >= NR:
                    return
                tile_ = stpool.tile([128, 2048], fp16, tag="st")
                # same SWDGE queue as hats: FIFO order == consumption order,
                # so the urgent state round is never starved by hats bulk
                nc.gpsimd.dma_start(tile_[:], state_d[r])
                st_tiles[r] = tile_
                next_st[0] += 1

            # state first: the first phase-A round must not queue behind the
            # 6MB hats prefetch burst (SDMA fair-shares rings at packet level)
            fetch_st()
            fetch_st()
            for _ in range(min(HB_BUFS - 1, NSEG)):
                fetch_seg()

            cp_i = [0]          # copy engine round robin

            def copy(out_ap, in_ap):
                # weight DVE slightly more than Act (Act also does tanh+DMA)
                k = cp_i[0] % 5
                cp_i[0] += 1
                if k in (0, 1, 3):
                    nc.vector.tensor_scalar_add(out=out_ap, in0=in_ap, scalar1=0.0)
                else:
                    nc.scalar.activation(out_ap, in_ap, Act.Copy)

            for r in range(NR):
                stb = st_tiles.pop(r)
                fetch_st()
                tp = psA.tile([128, 512], f32, tag="tp")
                for i in range(8):          # (hi/lo, c4) combos
                    c4 = i % 4
                    nc.tensor.matmul(
                        tp[:],
                        lhsT=wcat[:, i * 128:(i + 1) * 128],
                        rhs=stb[:, c4 * 512:(c4 + 1) * 512],
                        start=(i == 0), stop=(i == 7),
                    )
                thT4 = thpool.tile([128, 512], fp16, tag="thT4")
                nc.scalar.activation(thT4[:], tp[:], Act.Tanh, bias=bfs[:, 0:1])

                for b4 in range(4):
                    b = 4 * r + b4
                    thT = thT4[:, b4 * 128:(b4 + 1) * 128]
                    nc.tensor.matmul(u0acc[:, b:b + 1], lhsT=thT, rhs=wfp[:],
                                     start=True, stop=True)
                    blk = geom.blocks[b]
                    if blk is None:
                        continue
                    d, C, ng, W, gplace, ukoff = blk
                    LB = ng * W     # = 128*d
                    stage = sgpool.tile([128, STGW], f32, tag="stage")
                    for g in range(ng):
                        s, off = gplace[g]
                        while next_seg[0] <= s + (HB_BUFS - 2):
                            if next_seg[0] >= NSEG:
                                break
                            fetch_seg()
                        hs = hseg[s]
                        pk = psK.tile([128, 512], f32, tag="pk")
                        nc.tensor.matmul(
                            pk[0:C, 0:W],
                            lhsT=thT[:, g * C:(g + 1) * C],
                            rhs=hs[:, off:off + W],
                            start=True, stop=True)
                        copy(stage[0:C, g * W:(g + 1) * W], pk[0:C, 0:W])
                    sap = stage[:]
                    diag = AP(sap.tensor, sap.offset,
                              [(STGW + d, C), (W, ng), (1, d)])
                    nc.sync.dma_start(uk_d[0:1, ukoff:ukoff + 128 * d], diag)

            u0sb = ppool.tile([128, NBLK4], f32, tag="u0sb")
            nc.scalar.activation(u0sb[:], u0acc[:], Act.Sigmoid, bias=bfp[:, 0:1])
            nc.sync.dma_start(u0_d[:], u0sb[:])
    return []


def build_nc(geom):
    import concourse.bass as bass
    from concourse import mybir

    fp16 = mybir.dt.float16
    f32 = mybir.dt.float32

    nc = bass.Bass("TRN2", target_bir_lowering=False, debug=False)
    t = {
        "wcat": nc.dram_tensor("wcat", [128, 8 * 128], fp16,
                               kind="ExternalInput")[:],
        "bfs": nc.dram_tensor("bfs", [128, 1], fp16, kind="ExternalInput")[:],
        "wfp": nc.dram_tensor("wfp", [128, 1], fp16, kind="ExternalInput")[:],
        "bfp": nc.dram_tensor("bfp", [128, 1], f32, kind="ExternalInput")[:],
        "state_p": nc.dram_tensor("state_p", [geom.NR, 128, 2048], fp16,
                                  kind="ExternalInput"),
        "hats_p": nc.dram_tensor("hats_p", [geom.NSEG, 128, SEGCOLS], fp16,
                                 kind="ExternalInput"),
        "uk_o": nc.dram_tensor("uk_o", [1, geom.SUK], f32,
                               kind="ExternalOutput")[:],
        "u0_o": nc.dram_tensor("u0_o", [128, geom.NBLK4], f32,
                               kind="ExternalOutput")[:],
    }
    emit(nc, t, geom)
    split_multi_waits(nc)
    return nc


def split_multi_waits(nc):
    """This env's walrus encodes at most one sem wait per instruction; hoist
    extras onto standalone EventSemaphore insts immediately before."""
    import concourse.mybir as mybir
    n = 0
    for fn in nc.m.functions:
        for bb in fn.blocks:
            insts = list(bb.instructions)
            if not any(i.sync_info and len(i.sync_info.on_wait) > 1 for i in insts):
                continue
            out = []
            for inst in insts:
                si = inst.sync_info
                if si is not None and len(si.on_wait) > 1:
                    waits = list(si.on_wait)
                    for w in waits[:-1]:
                        n += 1
                        out.append(mybir.InstEventSemaphore(
                            name=f"splitw_{n}_{inst.name}",
                            engine=inst.engine, ins=[], outs=[],
                            sync_info=mybir.SyncInfo(on_wait=[w], on_update=[]),
                        ))
                    inst.sync_info = mybir.SyncInfo(
                        on_wait=[waits[-1]], on_update=list(si.on_update))
                out.append(inst)
            bb.instructions = out
    return n


def prep_inputs(state_HS, hats, seg, W_fs, b_fs, W_fp, b_fp):
    """Shard + pack. Returns (in_maps, geom, maps)."""
    state_HS = np.asarray(state_HS, dtype=np.float32)
    hats = np.asarray(hats, dtype=np.float32)
    seg = np.asarray(seg, dtype=np.int32)
    W_fs = np.asarray(W_fs, dtype=np.float32)
    b_fs = np.asarray(b_fs, dtype=np.float32)
    W_fp = np.asarray(W_fp, dtype=np.float32)
    b_fp = np.asarray(b_fp, dtype=np.float32)

    deg = np.bincount(seg, minlength=N_NODES).astype(np.int64)
    estart = np.concatenate([[0], np.cumsum(deg)[:-1]])

    orders = []
    deg_sorted = np.empty((NC, NPER), np.int64)
    for c in range(NC):
        nodes = np.arange(c * NPER, (c + 1) * NPER)
        o = np.lexsort((nodes, -deg[nodes]))
        orders.append(nodes[o])
        deg_sorted[c] = deg[nodes[o]]
    D = deg_sorted.max(axis=0)
    geom = Geom(D)

    # constants
    w_hi = W_fs.astype(np.float16)
    w_lo = (W_fs.astype(np.float64) - w_hi.astype(np.float64)).astype(np.float16)
    wcat = np.empty((128, 8, 128), dtype=np.float16)
    for hl, w in enumerate((w_hi, w_lo)):
        for c4 in range(4):
            wcat[:, hl * 4 + c4, :] = w[c4 * 128:(c4 + 1) * 128, :]
    wcat = wcat.reshape(128, 8 * 128)
    bfs_c = _f16(b_fs.reshape(128, 1))
    wfp_c = _f16(W_fp.reshape(128, 1))
    bfp_c = np.full((128, 1), float(b_fp[0]), dtype=np.float32)

    state16 = state_HS.astype(np.float16)
    hats16 = hats.astype(np.float16)

    in_maps, maps = [], []
    for c in range(NC):
        order = orders[c]
        ordp = np.concatenate(
            [order, np.full(geom.NCHUNK - NPER, order[-1], np.int64)])
        degp = np.concatenate(
            [deg_sorted[c], np.zeros(geom.NCHUNK - NPER, np.int64)])

        # state rounds [NR, 128, 2048]
        st_p = np.empty((geom.NR, 128, 2048), np.float16)
        for r in range(geom.NR):
            nodes512 = ordp[512 * r:512 * (r + 1)]
            arr = state16[nodes512]                      # [512, 512]
            st_p[r] = (arr.reshape(512, 4, 128)
                       .transpose(2, 1, 0).reshape(128, 2048))

        # hats segments [NSEG, 128, SEGCOLS]
        hp = np.zeros((geom.NSEG, 128, SEGCOLS), np.float16)
        for b in range(geom.NBLK4):
            blk = geom.blocks[b]
            if blk is None:
                continue
            d, C, ng, W, gplace, ukoff = blk
            ci = 128 * b + np.arange(128)                # chunk index [ng*C]
            nodesb = ordp[ci].reshape(ng, C)
            degb = degp[ci].reshape(ng, C)
            j = np.arange(d)
            e = estart[nodesb][:, :, None] + j[None, None, :]
            valid = j[None, None, :] < degb[:, :, None]
            eidx = np.where(valid, e, 0)
            vals = hats16[eidx]                          # [ng, C, d, 128]
            for g in range(ng):
                s, off = gplace[g]
                hp[s][:, off:off + W] = vals[g].reshape(W, 128).T
        in_maps.append({
            "wcat": wcat, "bfs": bfs_c, "wfp": wfp_c, "bfp": bfp_c,
            "state_p": st_p,
            "hats_p": np.ascontiguousarray(hp),
        })
        maps.append((ordp, degp, estart[ordp]))
    return in_maps, geom, maps


def assemble(results, geom, maps):
    out = np.empty(N_NODES + N_EDGES, dtype=np.float32)
    for c in range(NC):
        ordp, degp, e0p = maps[c]
        uk = np.asarray(results[c]["uk_o"]).reshape(-1)
        u0 = np.asarray(results[c]["u0_o"])              # [128, NBLK4]
        i = np.arange(NPER)
        out[ordp[:NPER]] = u0[i % 128, i // 128]
        # uk: per block, diag output order is [chunk-in-group, group, slot]
        srcs, dsts = [], []
        for b in range(geom.NBLK4):
            blk = geom.blocks[b]
            if blk is None:
                continue
            d, C, ng, W, gplace, ukoff = blk
            ci = 128 * b + np.arange(128)
            degb = degp[ci]
            cc = np.arange(128)                          # chunk-in-block
            g = cc // C
            r = cc % C
            j = np.arange(d)
            pos = ukoff + r[:, None] * (ng * d) + g[:, None] * d + j[None, :]
            valid = j[None, :] < degb[:, None]
            if not valid.any():
                continue
            # edge index needs global estart; recompute cheaply
            srcs.append(pos[valid])
            e0 = e0p[ci]
            dsts.append((e0[:, None] + j[None, :])[valid])
        if srcs:
            sp = np.concatenate(srcs)
            dp = np.concatenate(dsts)
            out[N_NODES + dp] = 1.0 / (1.0 + np.exp(-uk[sp]))
    return out


def kernel(state_HS, hats, seg, W_fs, b_fs, W_fp, b_fp):
    from concourse.bass_utils import run_bass_kernel_spmd
    in_maps, geom, maps = prep_inputs(
        state_HS, hats, seg, W_fs, b_fs, W_fp, b_fp)
    nc = build_nc(geom)
    res = run_bass_kernel_spmd(nc, in_maps, core_ids=list(range(NC)))
    return assemble(res.results, geom, maps)
